# revision 1
# baseline (speedup 1.0000x reference)
"""MiniSTU (spectral transform unit) Trainium2 kernel.

Math: out[b,l,o] = sum_k conv_causal(phi_k, x @ Mp[k])[l,o]
               + sum_k (-1)^(l-t)-weighted conv_causal(phi_k, x @ Mm[k])[l,o]

The FFT convolution of the reference is a causal linear convolution
(n=2048 >= 2L-1), computed here as block-Toeplitz matmuls on the tensor
engine.  The alternating-sign modulation of the minus branch depends only
on (l - t), so it folds entirely into the precomputed Toeplitz weights.

Sharding: the K=24 filters are split 3-per-core across 8 cores (every core
runs the identical program on its own filter slice); the host sums the 8
partial outputs.

Per-core schedule (two passes over output-column halves `oh`):
  for lb in 0..7:                       # output time-block, 128 rows
    Y[lb]  = xT-block @ Mcat            # projection, PSUM over i-chunks
    out[lb] = sum_{tb<=lb,k',sign} W[lb-tb,k',sign] @ Y[tb]   # PSUM accum
All matmuls use float32r (full-speed fp32 path, N>=256).
"""

import os
# Ask the runtime to reset cores on acquisition: recovers from a prior
# process leaving a core in NRT_EXEC_UNIT_UNRECOVERABLE state.
os.environ.setdefault("NEURON_RT_RESET_CORES", "1")

import numpy as np
import concourse.bacc as bacc
import concourse.mybir as mybir
from concourse.tile import TileContext
from concourse.bass_utils import run_bass_kernel_spmd

B, L, I, O, K = 4, 1024, 256, 256, 24
S = 128           # block size
NB = L // S       # 8 time blocks
KPC = 3           # filters per core
N_CORES = 8
F32 = mybir.dt.float32
F32R = mybir.dt.float32r

_cache = {}


def _build_program(reps=1):
    """reps>1 repeats the whole compute (timing experiments only)."""
    nc = bacc.Bacc()
    # [ic, i, b*NB*S]  (xT tiles: col = b*1024 + tb*128 + t)
    xt_d = nc.declare_dram_parameter("xt", [2, S, B * NB * S], F32R, isOutput=False)
    # [ic, i, 1536]    (col = oh*768 + sign*384 + kp*128 + o)
    m_d = nc.declare_dram_parameter("mcat", [2, S, 1536], F32R, isOutput=False)
    # [d, t, 768]      (col = kp*256 + sign*128 + l)
    w_d = nc.declare_dram_parameter("w", [NB, S, 768], F32R, isOutput=False)
    # [oh, lb, l, b*128]  (col = b*128 + o)
    out_d = nc.declare_dram_parameter("out", [2, NB, S, B * S], F32, isOutput=True)

    with TileContext(nc) as tc:
        with tc.tile_pool(name="persist", bufs=1) as persist, \
             tc.tile_pool(name="ypool", bufs=NB + 1) as ypool, \
             tc.tile_pool(name="ostage", bufs=3) as ostage, \
             tc.tile_pool(name="pyp", bufs=3, space="PSUM") as pyp, \
             tc.tile_pool(name="poutp", bufs=2, space="PSUM") as poutp:

            # Per-(ic,oh) M tiles, per-(ic,b) xT tiles, per-d W tiles: fine
            # DMA granularity so the first projection/conv only waits on the
            # chunks it reads.  Issue order = first-use order.
            m_sb = {}
            for oh in range(2):
                for ic in range(2):
                    for ch, w_ in ((0, 512), (1, 256)):
                        m_sb[ic, oh, ch] = persist.tile(
                            [S, w_], F32R, tag=f"m{ic}{oh}{ch}",
                            name=f"m_sb{ic}{oh}{ch}")
            xt_sb = {}
            for b in range(B):
                for ic in range(2):
                    for q in range(NB // 2):
                        xt_sb[ic, b, q] = persist.tile(
                            [S, 2 * S], F32R, tag=f"xt{ic}{b}{q}",
                            name=f"xt_sb{ic}{b}{q}")
            w_sb = {}
            for d in range(NB):
                w_sb[d] = persist.tile(
                    [S, 768], F32R, tag=f"w{d}", name=f"w_sb{d}")

            # First-use-critical loads on HWDGE in exact first-need order;
            # bulk streams on SWDGE (gpsimd).  xt is tiled per (ic, b,
            # lb-pair) so projection lb only waits on its own slices.
            def xt_dma(eng, ic, b, q):
                eng.dma_start(
                    out=xt_sb[ic, b, q][:],
                    in_=xt_d[ic, :, b * NB * S + q * 2 * S:
                             b * NB * S + (q + 1) * 2 * S])
            nc.sync.dma_start(out=m_sb[0, 0, 0][:], in_=m_d[0, :, 0:512])
            xt_dma(nc.sync, 0, 0, 0)
            nc.sync.dma_start(out=m_sb[1, 0, 0][:], in_=m_d[1, :, 0:512])
            xt_dma(nc.sync, 1, 0, 0)
            nc.sync.dma_start(out=m_sb[0, 0, 1][:], in_=m_d[0, :, 512:768])
            nc.sync.dma_start(out=m_sb[1, 0, 1][:], in_=m_d[1, :, 512:768])
            xt_dma(nc.sync, 0, 2, 0)
            xt_dma(nc.sync, 1, 2, 0)
            xt_dma(nc.gpsimd, 0, 1, 0)
            xt_dma(nc.gpsimd, 1, 1, 0)
            nc.gpsimd.dma_start(out=w_sb[0][:], in_=w_d[0])
            xt_dma(nc.gpsimd, 0, 3, 0)
            xt_dma(nc.gpsimd, 1, 3, 0)
            nc.gpsimd.dma_start(out=w_sb[1][:], in_=w_d[1])
            for q in range(1, NB // 2):
                for b in range(B):
                    for ic in range(2):
                        xt_dma(nc.gpsimd, ic, b, q)
                nc.gpsimd.dma_start(out=w_sb[2 * q][:], in_=w_d[2 * q])
                nc.gpsimd.dma_start(out=w_sb[2 * q + 1][:], in_=w_d[2 * q + 1])
            for ic in range(2):
                for ch, lo, hi in ((0, 768, 1280), (1, 1280, 1536)):
                    nc.gpsimd.dma_start(out=m_sb[ic, 1, ch][:],
                                        in_=m_d[ic, :, lo:hi])

            warm = persist.tile([S, 256], mybir.dt.bfloat16, tag="warm",
                                name="warm_sb")
            nc.vector.memset(warm[:], 0.0)
            for wi in range(8):
                pwarm = poutp.tile([S, 256], F32, tag="pout", name=f"pwarm{wi}")
                nc.tensor.matmul(pwarm[:], lhsT=warm[:, 0:128],
                                 rhs=warm[:], start=True, stop=True)

            for rep in range(reps):
                for oh in range(2):
                    y_tiles = []
                    for lb in range(NB):
                        # ---- projection: Y[lb] for all b, both signs ----
                        y_t = ypool.tile([S, 3072], F32R, tag="y",
                                         name=f"y_{rep}_{oh}_{lb}")
                        y_tiles.append(y_t)
                        for b in range(B):
                            py = pyp.tile([S, 768], F32, tag="py",
                                          name=f"py_{rep}_{oh}_{lb}_{b}")
                            for c0, c1 in ((0, 512), (512, 768)):
                                for ic in range(2):
                                    nc.tensor.matmul(
                                        py[:, c0:c1],
                                        lhsT=xt_sb[ic, b, lb // 2]
                                             [:, (lb % 2) * S:(lb % 2 + 1) * S],
                                        rhs=m_sb[ic, oh, 0 if c0 == 0 else 1][:, 0:c1 - c0],
                                        start=(ic == 0), stop=(ic == 1),
                                    )
                            # scatter (sign,kp,o) -> y col kp*1024+sign*512+b*128+o
                            src = py[:].rearrange("p (s k o) -> p k s o",
                                                  s=2, k=KPC)
                            dst = y_t[:].rearrange(
                                "p (k s bb o) -> p k s bb o",
                                k=KPC, s=2, bb=B, o=S)[:, :, :, b, :]
                            if b % 2 == 0:
                                nc.vector.tensor_copy(out=dst, in_=src)
                            else:
                                nc.scalar.copy(out=dst, in_=src)
                        # ---- conv accumulation into out block lb ----
                        pout = poutp.tile([S, 512], F32, tag="pout",
                                          name=f"pout_{rep}_{oh}_{lb}")
                        n_mm = 6 * (lb + 1)
                        i_mm = 0
                        for tb in range(lb + 1):
                            d = lb - tb
                            for kp in range(KPC):
                                for sg in range(2):
                                    nc.tensor.matmul(
                                        pout[:, 0:512],
                                        lhsT=w_sb[d][:, kp * 256 + sg * 128:
                                                     kp * 256 + sg * 128 + 128],
                                        rhs=y_tiles[tb][:, kp * 1024 + sg * 512:
                                                        kp * 1024 + sg * 512 + 512],
                                        start=(i_mm == 0),
                                        stop=(i_mm == n_mm - 1),
                                    )
                                    i_mm += 1
                        ost = ostage.tile([S, 512], F32, tag="ost",
                                          name=f"ost_{rep}_{oh}_{lb}")
                        nc.vector.tensor_copy(out=ost[:], in_=pout[:])
                        nc.sync.dma_start(out=out_d[oh, lb], in_=ost[:])
    nc.finalize()
    return nc


def _host_pack(x, phi, M_phi_plus, M_phi_minus):
    """Build host-side packed arrays; returns (xt, mcat_percore, w_percore)."""
    x = np.ascontiguousarray(x, dtype=np.float32)
    phi = np.ascontiguousarray(phi, dtype=np.float32)
    Mp = np.ascontiguousarray(M_phi_plus, dtype=np.float32)
    Mm = np.ascontiguousarray(M_phi_minus, dtype=np.float32)

    # xt[ic, i, b*1024 + tb*128 + t] = x[b, tb*128+t, ic*128+i]
    xt = np.ascontiguousarray(x.transpose(2, 0, 1).reshape(2, S, B * L))

    # Toeplitz blocks: base = d*128 + l - t
    tt = np.arange(S)
    ll = np.arange(S)
    arg = ll[None, :] - tt[:, None]                      # [t, l]
    base = arg[None, :, :] + (np.arange(NB) * S)[:, None, None]  # [d, t, l]
    valid = base >= 0
    idx = np.clip(base, 0, L - 1)
    Wp = np.where(valid[..., None], phi[idx], 0.0)       # [d, t, l, K]
    par = np.where(base % 2 == 0, 1.0, -1.0).astype(np.float32)
    Wm = Wp * par[..., None]
    # per-core w[d, t, kp*256 + sign*128 + l]
    w_cores = []
    for c in range(N_CORES):
        ks = slice(KPC * c, KPC * (c + 1))
        wc = np.stack([Wp[..., ks], Wm[..., ks]], axis=-1)  # [d,t,l,kp,2]
        wc = wc.transpose(0, 1, 3, 4, 2).reshape(NB, S, 768)
        w_cores.append(np.ascontiguousarray(wc.astype(np.float32)))

    # mcat[ic, i, oh*768 + sign*384 + kp*128 + o] = M_sign[kg, ic*128+i, oh*128+o]
    m_cores = []
    for c in range(N_CORES):
        ks = slice(KPC * c, KPC * (c + 1))
        mp = Mp[ks].reshape(KPC, 2, S, 2, S)   # [kp, ic, i, oh, o]
        mm = Mm[ks].reshape(KPC, 2, S, 2, S)
        mc = np.stack([mp, mm], axis=0)        # [sign, kp, ic, i, oh, o]
        mc = mc.transpose(2, 3, 4, 0, 1, 5).reshape(2, S, 1536)
        m_cores.append(np.ascontiguousarray(mc.astype(np.float32)))

    return xt, m_cores, w_cores


def kernel(x, phi, M_phi_plus, M_phi_minus):
    if "nc" not in _cache:
        _cache["nc"] = _build_program()
    nc = _cache["nc"]

    xt, m_cores, w_cores = _host_pack(x, phi, M_phi_plus, M_phi_minus)
    in_maps = [
        {"xt": xt, "mcat": m_cores[c], "w": w_cores[c]}
        for c in range(N_CORES)
    ]
    res = None
    last_err = None
    for attempt in range(3):
        try:
            res = run_bass_kernel_spmd(nc, in_maps,
                                       core_ids=list(range(N_CORES)))
            break
        except Exception as e:  # transient device wedge: retry
            last_err = e
    if res is None:
        raise last_err
    # out[oh, lb, l, b*128+o]; sum over cores, then reassemble [b, l, o]
    acc = np.zeros((2, NB, S, B * S), dtype=np.float64)
    for om in res.results:
        acc += om["out"]
    acc = acc.reshape(2, NB, S, B, S)           # [oh, lb, l, b, o]
    out = acc.transpose(3, 1, 2, 0, 4).reshape(B, L, O)
    return np.ascontiguousarray(out.astype(np.float32))



# revision 2
# speedup vs baseline: 1.0766x; 1.0766x over previous
"""MiniSTU (spectral transform unit) Trainium2 kernel — parity-factorized.

Math: out[b,l,o] = sum_k sum_{d<=l} phi_k[d] * ( u_k[l-d,o] if d even
                                                 else v_k[l-d,o] )
with u_k = x @ (Mp_k + Mm_k), v_k = x @ (Mp_k - Mm_k).

This is the exact parity factorization of the reference's plus/minus
branches: conv(phi)y+ + conv(phi~)y- = conv(even lags)(y+ + y-) +
conv(odd lags)(y+ - y-), and even/odd lags only couple matching/opposite
time parities.  The convolution therefore splits into four half-length
(512-sample) causal convolutions per filter — half the tensor-engine
work of the direct block-Toeplitz form.

Sharding: K=24 filters split 3-per-core across 8 cores; host sums the 8
partial outputs.

Per-core schedule (two passes over output-column halves `oh`):
  pairs qq = 0..3 on the half-grid (each pair = even+odd parity block):
    proj(qq): Y[p=0,qq], Y[p=1,qq] = xT-parity-block @ [Msum|Mdiff]
    conv(mb): out_e[mb] += A[mb-tb]^T u_e[tb] + B[mb-tb]^T v_o[tb]
              out_o[mb] += A[mb-tb]^T u_o[tb] + B'[mb-tb]^T v_e[tb]
  software-pipelined: proj0 proj1 conv0 proj2 conv1 proj3 conv3off
  conv2 conv3diag (keeps PE fed; only the d=0 taps of the last pair
  wait on the final projection).
All matmuls use float32r (full-speed fp32, N>=256).
"""

import os
os.environ.setdefault("NEURON_RT_RESET_CORES", "1")

import numpy as np
import concourse.bacc as bacc
import concourse.mybir as mybir
from concourse.tile import TileContext
from concourse.bass_utils import run_bass_kernel_spmd

B, L, I, O, K = 4, 1024, 256, 256, 24
S = 128           # block size
NBH = 4           # half-grid blocks (512 = 4*128)
KPC = 3           # filters per core
N_CORES = 8
F32 = mybir.dt.float32
F32R = mybir.dt.float32r
BF16 = mybir.dt.bfloat16

_cache = {}


def _build_program(reps=1):
    nc = bacc.Bacc()
    # [ic, i, b*1024 + j*128 + m], j = 2*mb + p, t = 2*(mb*128+m) + p
    xt_d = nc.declare_dram_parameter("xt", [2, S, B * L], BF16, isOutput=False)
    # [ic, i, oh*768 + w*384 + kp*128 + o]   (w=0: Mp+Mm, w=1: Mp-Mm)
    m_d = nc.declare_dram_parameter("mcat", [2, S, 1536], BF16, isOutput=False)
    # [d, t', kp*384 + typ*128 + m']  (typ 0=A even taps, 1=B, 2=B')
    w_d = nc.declare_dram_parameter("w", [NBH, S, 1152], BF16, isOutput=False)
    # [oh, p*4+mb, m', b*128+o]
    out_d = nc.declare_dram_parameter("out", [2, 2 * NBH, S, B * S], BF16,
                                      isOutput=True)

    with TileContext(nc) as tc:
        with tc.tile_pool(name="persist", bufs=1) as persist, \
             tc.tile_pool(name="ypool", bufs=2 * NBH + 1) as ypool, \
             tc.tile_pool(name="ostage", bufs=3) as ostage, \
             tc.tile_pool(name="pya", bufs=2, space="PSUM") as pya, \
             tc.tile_pool(name="pyb", bufs=2, space="PSUM") as pyb, \
             tc.tile_pool(name="poutp", bufs=4, space="PSUM") as poutp:

            # m_sb[ic, oh] holds all 768 projection columns for that half
            m_sb = {}
            for oh in range(2):
                for ic in range(2):
                    m_sb[ic, oh] = persist.tile(
                        [S, 768], BF16, tag=f"m{ic}{oh}",
                        name=f"m_sb{ic}{oh}")
            # xt_sb[ic, b, h] covers half-grid pairs 2h, 2h+1 (512 cols)
            xt_sb = {}
            for b in range(B):
                for ic in range(2):
                    for h in range(2):
                        xt_sb[ic, b, h] = persist.tile(
                            [S, 4 * S], BF16, tag=f"xt{ic}{b}{h}",
                            name=f"xt_sb{ic}{b}{h}")
            w_sb = {}
            for d in range(NBH):
                w_sb[d] = persist.tile(
                    [S, 1152], BF16, tag=f"w{d}", name=f"w_sb{d}")

            warm = persist.tile([S, 512], mybir.dt.bfloat16, tag="warm",
                                name="warm_sb")
            nc.vector.memset(warm[:, 0:128], 0.0)
            nc.gpsimd.memset(warm[:, 128:512], 0.0)

            # ---- DMA issue.  HWDGE (sync) is a shared serial device with
            # a flat 625ns cost per transfer: few, large transfers, first-
            # use order.  Bulk second halves ride SWDGE (gpsimd/Pool). ----
            def xt_dma(eng, ic, b, h):
                eng.dma_start(
                    out=xt_sb[ic, b, h][:],
                    in_=xt_d[ic, :, b * L + h * 4 * S:
                             b * L + (h + 1) * 4 * S])
            nc.sync.dma_start(out=m_sb[0, 0][:], in_=m_d[0, :, 0:768])
            xt_dma(nc.sync, 0, 0, 0)
            nc.sync.dma_start(out=m_sb[1, 0][:], in_=m_d[1, :, 0:768])
            xt_dma(nc.sync, 1, 0, 0)
            xt_dma(nc.sync, 0, 3, 0)
            xt_dma(nc.sync, 1, 3, 0)
            for b in (1, 2):
                for ic in range(2):
                    xt_dma(nc.gpsimd, ic, b, 0)
            nc.gpsimd.dma_start(out=w_sb[0][:], in_=w_d[0])
            for b in range(B):
                for ic in range(2):
                    xt_dma(nc.gpsimd, ic, b, 1)
            nc.gpsimd.dma_start(out=w_sb[1][:], in_=w_d[1])
            nc.gpsimd.dma_start(out=w_sb[2][:], in_=w_d[2])
            nc.gpsimd.dma_start(out=w_sb[3][:], in_=w_d[3])
            for ic in range(2):
                nc.gpsimd.dma_start(out=m_sb[ic, 1][:],
                                    in_=m_d[ic, :, 768:1536])

            # ---- PE warm-up: >3us of dummy matmuls under the DMA shadow ----
            for wi in range(9):
                pwarm = poutp.tile([S, 512], F32, tag="pout",
                                   name=f"pwarm{wi}")
                nc.tensor.matmul(
                    pwarm[:, 0:128] if wi < 2 else pwarm[:],
                    lhsT=warm[:, 0:128],
                    rhs=warm[:, 0:128] if wi < 2 else warm[:],
                    start=True, stop=True)

            for rep in range(reps):
                for oh in range(2):
                    # y[(p, mb)] tiles, cols = kp*1024 + w*512 + b*128 + o
                    y = {}

                    def proj(qq):
                        for jj in range(2):      # jj = parity p
                            y_t = ypool.tile([S, 3072], BF16, tag="y",
                                             name=f"y_{rep}_{oh}_{qq}_{jj}")
                            y[jj, qq] = y_t
                            for b in range(B):
                                xcol = (qq % 2) * 2 * S + jj * S
                                dst = y_t[:].rearrange(
                                    "p (k w bb o) -> p k w bb o",
                                    k=KPC, w=2, bb=B, o=S)[:, :, :, b, :]
                                # w=0 / w=1 projection chunks go to separate
                                # PSUM tiles so each frees at its own scatter
                                for w_i, pool_, eng in (
                                        (0, pya, nc.vector),
                                        (1, pyb, nc.scalar)):
                                    py = pool_.tile(
                                        [S, 384], F32, tag=f"py{w_i}",
                                        name=f"py{w_i}_{rep}_{oh}_{qq}"
                                             f"_{jj}_{b}")
                                    for ic in range(2):
                                        nc.tensor.matmul(
                                            py[:],
                                            lhsT=xt_sb[ic, b, qq // 2]
                                                 [:, xcol:xcol + S],
                                            rhs=m_sb[ic, oh]
                                                [:, w_i * 384:
                                                 w_i * 384 + 384],
                                            start=(ic == 0), stop=(ic == 1),
                                        )
                                    src = py[:].rearrange(
                                        "p (k o) -> p k o", k=KPC)
                                    d_w = dst[:, :, w_i]
                                    if eng is nc.vector:
                                        eng.tensor_copy(out=d_w, in_=src)
                                    else:
                                        eng.copy(out=d_w, in_=src)

                    pout = {}

                    def conv_mms(mb, tbs, split=False):
                        """Emit conv matmuls of out-pair mb for source
                        blocks tbs.  p=0 (even outs): A taps on u_e plus
                        B taps on v_o; p=1: A on u_o plus B' on v_e.
                        split=True runs separate column-half PSUM groups
                        (used for the final pair so stores pipeline)."""
                        halves = ((0, 256), (256, 512)) if split \
                            else ((0, 512),)
                        for p in range(2):
                            if (p, mb) not in pout:
                                pout[p, mb] = poutp.tile(
                                    [S, 512], F32, tag="pout",
                                    name=f"pout_{rep}_{oh}_{p}_{mb}")
                            po = pout[p, mb]
                            n_all = 2 * KPC * (mb + 1)
                            done = 2 * KPC * tbs[0]
                            for h0, h1 in halves:
                                i_mm = done
                                for tb in tbs:
                                    d = mb - tb
                                    for kp in range(KPC):
                                        for typ_i in range(2):
                                            if p == 0:
                                                typ = 0 if typ_i == 0 else 1
                                                src_p = typ_i
                                                src_w = typ_i
                                            else:
                                                typ = 0 if typ_i == 0 else 2
                                                src_p = 1 - typ_i
                                                src_w = typ_i
                                            c = kp * 1024 + src_w * 512
                                            nc.tensor.matmul(
                                                po[:, h0:h1],
                                                lhsT=w_sb[d][:, kp * 384
                                                             + typ * S:
                                                             kp * 384
                                                             + typ * S + S],
                                                rhs=y[src_p, tb]
                                                    [:, c + h0:c + h1],
                                                start=(i_mm == 0),
                                                stop=(i_mm == n_all - 1),
                                            )
                                            i_mm += 1

                    def conv_out(mb, split=False, tail=False):
                        halves = ((0, 256), (256, 512)) if split \
                            else ((0, 512),)
                        for p in range(2):
                            po = pout[p, mb]
                            ost = ostage.tile([S, 512], BF16, tag="ost",
                                              name=f"ost_{rep}_{oh}_{p}_{mb}")
                            for hi, (h0, h1) in enumerate(halves):
                                alt = (p + hi) % 2 == 0
                                if alt:
                                    nc.vector.tensor_copy(
                                        out=ost[:, h0:h1], in_=po[:, h0:h1])
                                else:
                                    nc.scalar.copy(out=ost[:, h0:h1],
                                                   in_=po[:, h0:h1])
                                # SWDGE generation is 1us on Pool; keep it
                                # out of the end-of-kernel critical path
                                q = nc.sync if (tail or alt) else nc.gpsimd
                                q.dma_start(
                                    out=out_d[oh, p * NBH + mb][:, h0:h1],
                                    in_=ost[:, h0:h1])
                            del pout[p, mb]

                    proj(0)
                    proj(1)
                    proj(2)
                    conv_mms(1, [0, 1]); conv_out(1)
                    proj(3)
                    conv_mms(2, [0, 1, 2]); conv_out(2)
                    conv_mms(3, [0, 1, 2])     # off-diagonal taps
                    conv_mms(3, [3]); conv_out(3, tail=True)
                    # cheapest block last: its deps were ready long ago, so
                    # the final stores trail only a 12-matmul group
                    conv_mms(0, [0], split=True)
                    conv_out(0, split=True, tail=True)
    nc.finalize()
    return nc


def _host_pack(x, phi, M_phi_plus, M_phi_minus):
    x = np.ascontiguousarray(x, dtype=np.float32)
    phi = np.ascontiguousarray(phi, dtype=np.float32)
    Mp = np.ascontiguousarray(M_phi_plus, dtype=np.float32)
    Mm = np.ascontiguousarray(M_phi_minus, dtype=np.float32)

    # parity-major time permutation: col j*128+m -> t = 2*(mb*128+m)+p
    mb_ = np.repeat(np.arange(NBH), 2)          # j -> mb
    p_ = np.tile(np.arange(2), NBH)             # j -> p
    m_ = np.arange(S)
    tidx = (2 * (mb_[:, None] * S + m_[None, :]) + p_[:, None]).reshape(-1)
    import ml_dtypes
    bf16 = ml_dtypes.bfloat16
    xr = x.transpose(2, 0, 1)[:, :, tidx]       # [I, B, 1024]
    xt = np.ascontiguousarray(
        xr.reshape(2, S, B, L).reshape(2, S, B * L).astype(bf16))

    Ms = Mp + Mm
    Md = Mp - Mm
    phi_e = phi[0::2]                            # [512, K]
    phi_o = phi[1::2]

    dd = np.arange(NBH)
    base = (dd[:, None, None] * S + np.arange(S)[None, None, :]
            - np.arange(S)[None, :, None])       # [d, t', m']
    idx = np.clip(base, 0, 511)
    idxm1 = np.clip(base - 1, 0, 511)

    m_cores, w_cores = [], []
    for c in range(N_CORES):
        ks = slice(KPC * c, KPC * (c + 1))
        msd = np.stack([Ms[ks], Md[ks]], axis=0)      # [w, kp, I, O]
        msd = msd.reshape(2, KPC, 2, S, 2, S)         # [w, kp, ic, i, oh, o]
        mc = msd.transpose(2, 3, 4, 0, 1, 5).reshape(2, S, 1536)
        m_cores.append(np.ascontiguousarray(mc.astype(bf16)))

        wc = np.zeros((NBH, S, 1152), dtype=np.float32)
        for kp in range(KPC):
            k = KPC * c + kp
            A = np.where(base >= 0, phi_e[idx, k], 0.0)
            Bt = np.where(base - 1 >= 0, phi_o[idxm1, k], 0.0)
            Bp = np.where(base >= 0, phi_o[idx, k], 0.0)
            wc[:, :, kp * 384 + 0 * S:kp * 384 + 1 * S] = A
            wc[:, :, kp * 384 + 1 * S:kp * 384 + 2 * S] = Bt
            wc[:, :, kp * 384 + 2 * S:kp * 384 + 3 * S] = Bp
        w_cores.append(np.ascontiguousarray(wc.astype(bf16)))

    return xt, m_cores, w_cores


def kernel(x, phi, M_phi_plus, M_phi_minus):
    if "nc" not in _cache:
        _cache["nc"] = _build_program()
    nc = _cache["nc"]

    xt, m_cores, w_cores = _host_pack(x, phi, M_phi_plus, M_phi_minus)
    in_maps = [
        {"xt": xt, "mcat": m_cores[c], "w": w_cores[c]}
        for c in range(N_CORES)
    ]
    res = None
    last_err = None
    for attempt in range(3):
        try:
            res = run_bass_kernel_spmd(nc, in_maps,
                                       core_ids=list(range(N_CORES)))
            break
        except Exception as e:
            last_err = e
    if res is None:
        raise last_err
    # out[oh, p*4+mb, m, b*128+o]; sum over cores, reassemble [b, l, o]
    acc = np.zeros((2, 2 * NBH, S, B * S), dtype=np.float64)
    for om in res.results:
        acc += np.asarray(om["out"]).astype(np.float64)
    acc = acc.reshape(2, 2, NBH, S, B, S)       # [oh, p, mb, m, b, o]
    half = acc.transpose(4, 1, 2, 3, 0, 5)       # [b, p, mb, m, oh, o]
    half = half.reshape(B, 2, L // 2, O)         # [b, p, lhalf, o]
    out = np.empty((B, L, O), dtype=np.float64)
    out[:, 0::2] = half[:, 0]
    out[:, 1::2] = half[:, 1]
    return np.ascontiguousarray(out.astype(np.float32))


# revision 3
# speedup vs baseline: 1.0855x; 1.0083x over previous
"""MiniSTU (spectral transform unit) Trainium2 kernel — parity-factorized.

Math: out[b,l,o] = sum_k sum_{d<=l} phi_k[d] * ( u_k[l-d,o] if d even
                                                 else v_k[l-d,o] )
with u_k = x @ (Mp_k + Mm_k), v_k = x @ (Mp_k - Mm_k).

This is the exact parity factorization of the reference's plus/minus
branches: conv(phi)y+ + conv(phi~)y- = conv(even lags)(y+ + y-) +
conv(odd lags)(y+ - y-), and even/odd lags only couple matching/opposite
time parities.  The convolution therefore splits into four half-length
(512-sample) causal convolutions per filter — half the tensor-engine
work of the direct block-Toeplitz form.

Sharding: K=24 filters split 3-per-core across 8 cores; host sums the 8
partial outputs.

Per-core schedule (two passes over output-column halves `oh`):
  pairs qq = 0..3 on the half-grid (each pair = even+odd parity block):
    proj(qq): Y[p=0,qq], Y[p=1,qq] = xT-parity-block @ [Msum|Mdiff]
    conv(mb): out_e[mb] += A[mb-tb]^T u_e[tb] + B[mb-tb]^T v_o[tb]
              out_o[mb] += A[mb-tb]^T u_o[tb] + B'[mb-tb]^T v_e[tb]
  software-pipelined: proj0 proj1 conv0 proj2 conv1 proj3 conv3off
  conv2 conv3diag (keeps PE fed; only the d=0 taps of the last pair
  wait on the final projection).
All matmuls use float32r (full-speed fp32, N>=256).
"""

import os
os.environ.setdefault("NEURON_RT_RESET_CORES", "1")

import numpy as np
import concourse.bacc as bacc
import concourse.mybir as mybir
from concourse.tile import TileContext
from concourse.bass_utils import run_bass_kernel_spmd

B, L, I, O, K = 4, 1024, 256, 256, 24
S = 128           # block size
NBH = 4           # half-grid blocks (512 = 4*128)
KPC = 3           # filters per core
N_CORES = 8
F32 = mybir.dt.float32
F32R = mybir.dt.float32r
BF16 = mybir.dt.bfloat16
F8 = mybir.dt.float8e4
DR = mybir.MatmulPerfMode.DoubleRow

# fp8 pre-scales (powers of two; product folded into the conv taps)
SX = 2.0 ** 4
SM = 2.0 ** 10
SCALE = SX * SM

_cache = {}


def _build_program(reps=1):
    nc = bacc.Bacc()
    # fp8 DoubleRow pair layout: [i', ic, col].  x and M are split into a
    # quantized main term plus quantized residuals (host-side), so the
    # three-term DR projection is fp8-fast yet bf16-accurate:
    #   y = Q(x)Q(M) + Q(dx)Q(M) + Q(x)Q(dM)
    # col = b*1024 + j*128 + m, j = 2*mb + p, t = 2*(mb*128+m) + p
    xq_d = nc.declare_dram_parameter("xq", [S, 2, B * L], F8, isOutput=False)
    dxq_d = nc.declare_dram_parameter("dxq", [S, 2, B * L], F8, isOutput=False)
    # col = oh*768 + w*384 + kp*128 + o   (w=0: Mp+Mm, w=1: Mp-Mm)
    mq_d = nc.declare_dram_parameter("mq", [S, 2, 1536], F8, isOutput=False)
    dmq_d = nc.declare_dram_parameter("dmq", [S, 2, 1536], F8, isOutput=False)
    # [d, t', kp*384 + typ*128 + m']  (typ 0=A even taps, 1=B, 2=B')
    w_d = nc.declare_dram_parameter("w", [NBH, S, 1152], BF16, isOutput=False)
    # [oh, p*4+mb, m', b*128+o]
    out_d = nc.declare_dram_parameter("out", [2, 2 * NBH, S, B * S], BF16,
                                      isOutput=True)

    with TileContext(nc) as tc:
        with tc.tile_pool(name="persist", bufs=1) as persist, \
             tc.tile_pool(name="ypool", bufs=2 * NBH + 1) as ypool, \
             tc.tile_pool(name="ostage", bufs=6) as ostage, \
             tc.tile_pool(name="pya", bufs=3, space="PSUM") as pya, \
             tc.tile_pool(name="pyb", bufs=2, space="PSUM") as pyb, \
             tc.tile_pool(name="poutp", bufs=3, space="PSUM") as poutp:

            # m_sb[t, oh] (t: 0=main, 1=residual), pair dim = ic
            m_sb = {}
            for oh in range(2):
                for t in range(2):
                    m_sb[t, oh] = persist.tile(
                        [S, 2, 768], F8, tag=f"m{t}{oh}",
                        name=f"m_sb{t}{oh}")
            # xt_sb[t, b, h] covers half-grid pairs 2h, 2h+1 (512 cols)
            xt_sb = {}
            for b in range(B):
                for t in range(2):
                    for h in range(2):
                        xt_sb[t, b, h] = persist.tile(
                            [S, 2, 4 * S], F8, tag=f"xt{t}{b}{h}",
                            name=f"xt_sb{t}{b}{h}")
            w_sb = {}
            for d in range(NBH):
                w_sb[d] = persist.tile(
                    [S, 1152], BF16, tag=f"w{d}", name=f"w_sb{d}")

            # warm-up operand: one fast 128-col memset, then narrow matmuls
            warm = persist.tile([S, S], mybir.dt.bfloat16, tag="warm",
                                name="warm_sb")
            nc.vector.memset(warm[:], 0.0)

            # ---- DMA issue.  HWDGE (sync) is a shared serial device with
            # a flat 625ns cost per transfer: few, large transfers, first-
            # use order.  Bulk second halves ride SWDGE (gpsimd/Pool). ----
            def xt_dma(eng, t, b, h):
                src = xq_d if t == 0 else dxq_d
                eng.dma_start(
                    out=xt_sb[t, b, h][:],
                    in_=src[:, :, b * L + h * 4 * S:
                            b * L + (h + 1) * 4 * S])
            nc.sync.dma_start(out=m_sb[0, 0][:], in_=mq_d[:, :, 0:768])
            xt_dma(nc.sync, 0, 0, 0)
            nc.sync.dma_start(out=m_sb[1, 0][:], in_=dmq_d[:, :, 0:768])
            xt_dma(nc.sync, 1, 0, 0)
            xt_dma(nc.sync, 0, 2, 0)
            xt_dma(nc.sync, 1, 2, 0)
            for b in (0, 3):
                for t in range(2):
                    xt_dma(nc.sync, t, b, 1)
            for t in range(2):
                xt_dma(nc.sync, t, 1, 1)
            for b in (1, 3):
                for t in range(2):
                    xt_dma(nc.gpsimd, t, b, 0)
            nc.gpsimd.dma_start(out=w_sb[0][:], in_=w_d[0])
            nc.gpsimd.dma_start(out=w_sb[1][:], in_=w_d[1])
            for t in range(2):
                xt_dma(nc.gpsimd, t, 2, 1)
            nc.gpsimd.dma_start(out=w_sb[2][:], in_=w_d[2])
            nc.gpsimd.dma_start(out=w_sb[3][:], in_=w_d[3])
            nc.gpsimd.dma_start(out=m_sb[0, 1][:], in_=mq_d[:, :, 768:1536])
            nc.gpsimd.dma_start(out=m_sb[1, 1][:], in_=dmq_d[:, :, 768:1536])

            # ---- PE warm-up: >3us of dummy matmuls under the DMA shadow ----
            for wi in range(30):
                pwarm = poutp.tile([S, 512], F32, tag="pout",
                                   name=f"pwarm{wi}")
                nc.tensor.matmul(pwarm[:, 0:128], lhsT=warm[:],
                                 rhs=warm[:], start=True, stop=True)

            for rep in range(reps):
                for oh in range(2):
                    # y[(p, mb)] tiles, cols = kp*1024 + w*512 + b*128 + o
                    y = {}

                    def proj(qq):
                        for jj in range(2):      # jj = parity p
                            y_t = ypool.tile([S, 3072], BF16, tag="y",
                                             name=f"y_{rep}_{oh}_{qq}_{jj}")
                            y[jj, qq] = y_t
                            for b in range(B):
                                xcol = (qq % 2) * 2 * S + jj * S
                                dst = y_t[:].rearrange(
                                    "p (k w bb o) -> p k w bb o",
                                    k=KPC, w=2, bb=B, o=S)[:, :, :, b, :]
                                # w=0 / w=1 projection chunks go to separate
                                # PSUM tiles so each frees at its own scatter
                                for w_i, pool_, eng in (
                                        (0, pya, nc.vector),
                                        (1, pyb, nc.scalar)):
                                    py = pool_.tile(
                                        [S, 384], F32, tag=f"py{w_i}",
                                        name=f"py{w_i}_{rep}_{oh}_{qq}"
                                             f"_{jj}_{b}")
                                    # 3-term compensated fp8 DoubleRow:
                                    # both ic halves contract in one mm
                                    for i_t, (tx, tm) in enumerate(
                                            ((0, 0), (1, 0), (0, 1))):
                                        nc.tensor.matmul(
                                            py[:],
                                            lhsT=xt_sb[tx, b, qq // 2]
                                                 [:, :, xcol:xcol + S],
                                            rhs=m_sb[tm, oh]
                                                [:, :, w_i * 384:
                                                 w_i * 384 + 384],
                                            start=(i_t == 0),
                                            stop=(i_t == 2),
                                            perf_mode=DR,
                                        )
                                    src = py[:].rearrange(
                                        "p (k o) -> p k o", k=KPC)
                                    d_w = dst[:, :, w_i]
                                    if eng is nc.vector:
                                        eng.tensor_copy(out=d_w, in_=src)
                                    else:
                                        eng.copy(out=d_w, in_=src)

                    pout = {}

                    def conv_mms(mb, tbs, split=False):
                        """Emit conv matmuls of out-pair mb for source
                        blocks tbs.  p=0 (even outs): A taps on u_e plus
                        B taps on v_o; p=1: A on u_o plus B' on v_e.
                        split=True runs p=1 as two column-half PSUM
                        tiles (final pair: stores pipeline)."""
                        for p in range(2):
                            halves = ((0, 256), (256, 512)) \
                                if split and p == 1 else ((0, 512),)
                            n_all = 2 * KPC * (mb + 1)
                            done = 2 * KPC * tbs[0]
                            for h0, h1 in halves:
                                key = (p, mb, h0)
                                if key not in pout:
                                    pout[key] = poutp.tile(
                                        [S, h1 - h0], F32, tag="pout",
                                        name=f"pout_{rep}_{oh}_{p}_{mb}"
                                             f"_{h0}")
                                po = pout[key]
                                i_mm = done
                                for tb in tbs:
                                    d = mb - tb
                                    for kp in range(KPC):
                                        for typ_i in range(2):
                                            if p == 0:
                                                typ = 0 if typ_i == 0 else 1
                                                src_p = typ_i
                                                src_w = typ_i
                                            else:
                                                typ = 0 if typ_i == 0 else 2
                                                src_p = 1 - typ_i
                                                src_w = typ_i
                                            c = kp * 1024 + src_w * 512
                                            nc.tensor.matmul(
                                                po[:],
                                                lhsT=w_sb[d][:, kp * 384
                                                             + typ * S:
                                                             kp * 384
                                                             + typ * S + S],
                                                rhs=y[src_p, tb]
                                                    [:, c + h0:c + h1],
                                                start=(i_mm == 0),
                                                stop=(i_mm == n_all - 1),
                                            )
                                            i_mm += 1

                    def conv_out(mb, split=False, tail=False):
                        for p in range(2):
                            halves = ((0, 256), (256, 512)) \
                                if split and p == 1 else ((0, 512),)
                            ost = ostage.tile([S, 512], BF16, tag="ost",
                                              name=f"ost_{rep}_{oh}_{p}_{mb}")
                            for hi, (h0, h1) in enumerate(halves):
                                po = pout.pop((p, mb, h0))
                                alt = (p + hi) % 2 == 0
                                if alt:
                                    nc.vector.tensor_copy(
                                        out=ost[:, h0:h1], in_=po[:])
                                else:
                                    nc.scalar.copy(out=ost[:, h0:h1],
                                                   in_=po[:])
                                # SWDGE generation is 1us on Pool; for the
                                # last stores only the p=1 halves ride the
                                # HWDGE critical path, p=0 goes parallel
                                if tail:
                                    q = nc.gpsimd if p == 0 else nc.sync
                                else:
                                    q = nc.sync if alt else nc.gpsimd
                                q.dma_start(
                                    out=out_d[oh, p * NBH + mb][:, h0:h1],
                                    in_=ost[:, h0:h1])

                    # conv matmuls are braided between projection pairs so
                    # the scatter engines (DVE/Act) catch up while PE runs
                    # convs; the cheapest block (mb=0) closes the pass so
                    # the final stores trail only a 12-matmul group
                    proj(0)
                    conv_mms(1, [0])
                    proj(1)
                    conv_mms(1, [1]); conv_out(1)
                    proj(2)
                    conv_mms(2, [0, 1])
                    proj(3)
                    conv_mms(2, [2]); conv_out(2)
                    conv_mms(3, [0, 1, 2])     # off-diagonal taps
                    conv_mms(3, [3]); conv_out(3)
                    conv_mms(0, [0], split=True)
                    conv_out(0, split=True, tail=True)
    nc.finalize()
    return nc


def _host_pack(x, phi, M_phi_plus, M_phi_minus):
    x = np.ascontiguousarray(x, dtype=np.float32)
    phi = np.ascontiguousarray(phi, dtype=np.float32)
    Mp = np.ascontiguousarray(M_phi_plus, dtype=np.float32)
    Mm = np.ascontiguousarray(M_phi_minus, dtype=np.float32)

    # parity-major time permutation: col j*128+m -> t = 2*(mb*128+m)+p
    mb_ = np.repeat(np.arange(NBH), 2)          # j -> mb
    p_ = np.tile(np.arange(2), NBH)             # j -> p
    m_ = np.arange(S)
    tidx = (2 * (mb_[:, None] * S + m_[None, :]) + p_[:, None]).reshape(-1)
    import ml_dtypes
    bf16 = ml_dtypes.bfloat16
    f8 = ml_dtypes.float8_e4m3
    xr = x.transpose(2, 0, 1)[:, :, tidx] * SX  # [I, B, 1024], prescaled
    # [i', ic, col] with both ic halves in the DoubleRow pair dim
    xr = xr.reshape(2, S, B * L).transpose(1, 0, 2)
    xq = xr.astype(f8)
    dxq = (xr - xq.astype(np.float32)).astype(f8)
    xq = np.ascontiguousarray(xq)
    dxq = np.ascontiguousarray(dxq)

    Ms = Mp + Mm
    Md = Mp - Mm
    phi_e = phi[0::2]                            # [512, K]
    phi_o = phi[1::2]

    dd = np.arange(NBH)
    base = (dd[:, None, None] * S + np.arange(S)[None, None, :]
            - np.arange(S)[None, :, None])       # [d, t', m']
    idx = np.clip(base, 0, 511)
    idxm1 = np.clip(base - 1, 0, 511)

    m_cores, dm_cores, w_cores = [], [], []
    for c in range(N_CORES):
        ks = slice(KPC * c, KPC * (c + 1))
        msd = np.stack([Ms[ks], Md[ks]], axis=0) * SM  # [w, kp, I, O]
        msd = msd.reshape(2, KPC, 2, S, 2, S)          # [w, kp, ic, i, oh, o]
        # [i', ic, (oh, w, kp, o)]
        mc = msd.transpose(3, 2, 4, 0, 1, 5).reshape(S, 2, 1536)
        mcq = mc.astype(f8)
        dmcq = (mc - mcq.astype(np.float32)).astype(f8)
        m_cores.append(np.ascontiguousarray(mcq))
        dm_cores.append(np.ascontiguousarray(dmcq))

        wc = np.zeros((NBH, S, 1152), dtype=np.float32)
        for kp in range(KPC):
            k = KPC * c + kp
            A = np.where(base >= 0, phi_e[idx, k], 0.0)
            Bt = np.where(base - 1 >= 0, phi_o[idxm1, k], 0.0)
            Bp = np.where(base >= 0, phi_o[idx, k], 0.0)
            wc[:, :, kp * 384 + 0 * S:kp * 384 + 1 * S] = A
            wc[:, :, kp * 384 + 1 * S:kp * 384 + 2 * S] = Bt
            wc[:, :, kp * 384 + 2 * S:kp * 384 + 3 * S] = Bp
        # the fp8 prescale of x and M is folded back out here
        w_cores.append(np.ascontiguousarray((wc / SCALE).astype(bf16)))

    return xq, dxq, m_cores, dm_cores, w_cores


def kernel(x, phi, M_phi_plus, M_phi_minus):
    if "nc" not in _cache:
        _cache["nc"] = _build_program()
    nc = _cache["nc"]

    xq, dxq, m_cores, dm_cores, w_cores = _host_pack(
        x, phi, M_phi_plus, M_phi_minus)
    in_maps = [
        {"xq": xq, "dxq": dxq, "mq": m_cores[c], "dmq": dm_cores[c],
         "w": w_cores[c]}
        for c in range(N_CORES)
    ]
    res = None
    last_err = None
    for attempt in range(3):
        try:
            res = run_bass_kernel_spmd(nc, in_maps,
                                       core_ids=list(range(N_CORES)))
            break
        except Exception as e:
            last_err = e
    if res is None:
        raise last_err
    # out[oh, p*4+mb, m, b*128+o]; sum over cores, reassemble [b, l, o]
    acc = np.zeros((2, 2 * NBH, S, B * S), dtype=np.float64)
    for om in res.results:
        acc += np.asarray(om["out"]).astype(np.float64)
    acc = acc.reshape(2, 2, NBH, S, B, S)       # [oh, p, mb, m, b, o]
    half = acc.transpose(4, 1, 2, 3, 0, 5)       # [b, p, mb, m, oh, o]
    half = half.reshape(B, 2, L // 2, O)         # [b, p, lhalf, o]
    out = np.empty((B, L, O), dtype=np.float64)
    out[:, 0::2] = half[:, 0]
    out[:, 1::2] = half[:, 1]
    return np.ascontiguousarray(out.astype(np.float32))
